# revision 5
# baseline (speedup 1.0000x reference)
"""MoE classifier kernel for Trainium2 (8 NeuronCores, expert-parallel).

Strategy: the router (B x D @ D x E) + top-2 + softmax runs on host via
jax-CPU (bit-matching the reference); tokens are dispatched to their top-2
experts (all-to-all emulated by the host gather), each of the 8 cores runs
one expert's MLP (matmul1 -> LayerNorm -> GELU -> matmul2, gate-prob applied
on device), and the host scatter-adds the two contributions back.

Matmuls use the fp32r PE path (4-byte fp32 operands streamed at 1 cyc/row).
"""

import numpy as np

B, D, H, C, E, K = 16384, 1024, 4096, 512, 8, 2
LN_EPS = 1e-5
G = 2  # token-tiles per w1-streaming group

_cache = {}


def _build(n_tiles, use_b1, use_ln_affine):
    """Build + compile the per-core expert program. n_tiles = token tiles."""
    import concourse.bacc as bacc
    import concourse.mybir as mybir
    from concourse.tile import TileContext
    from concourse.masks import make_identity

    f32 = mybir.dt.float32
    f32r = mybir.dt.float32r
    AF = mybir.ActivationFunctionType
    N = n_tiles * 128

    nc = bacc.Bacc()
    xT = nc.declare_dram_parameter("xT", [D, N], f32, isOutput=False)
    w1 = nc.declare_dram_parameter("w1", [D, H], f32, isOutput=False)
    w2 = nc.declare_dram_parameter("w2", [H, C], f32, isOutput=False)
    prob = nc.declare_dram_parameter("prob", [N], f32, isOutput=False)
    if use_b1:
        b1 = nc.declare_dram_parameter("b1", [H], f32, isOutput=False)
    if use_ln_affine:
        ln_g = nc.declare_dram_parameter("ln_g", [H], f32, isOutput=False)
        ln_b = nc.declare_dram_parameter("ln_b", [H], f32, isOutput=False)
    y = nc.declare_dram_parameter("y", [N, C], f32, isOutput=True)

    xT_r = xT.rearrange("(k p) n -> p k n", p=128)      # [128, 8, N]
    w1_r = w1.rearrange("(k p) h -> p k h", p=128)      # [128, 8, H]
    w2_r = w2.rearrange("(k p) c -> p k c", p=128)      # [128, 32, C]
    prob_r = prob.rearrange("(t p) -> p t", p=128)      # [128, T]
    y_r = y.rearrange("(t p) c -> t p c", p=128)        # [T, 128, C]

    HJ = 8            # H chunks of 512 for matmul1
    KD = D // 128     # 8 k-tiles for matmul1
    KH = H // 128     # 32 k-tiles for matmul2

    with TileContext(nc) as tc:
        with (
            tc.tile_pool(name="const", bufs=1) as const,
            tc.tile_pool(name="w1p", bufs=2) as w1p,
            tc.tile_pool(name="xp", bufs=2 * G + 1) as xp,
            tc.tile_pool(name="hp", bufs=G + 1) as hp,
            tc.tile_pool(name="stp", bufs=G + 1) as stp,
            tc.tile_pool(name="utp", bufs=2) as utp,
            tc.tile_pool(name="yp", bufs=2) as ypool,
            tc.tile_pool(name="hps", bufs=3, space="PSUM") as hps,
            tc.tile_pool(name="tps", bufs=2, space="PSUM") as tps,
            tc.tile_pool(name="yps", bufs=2, space="PSUM") as yps,
        ):
            w2s = const.tile([128, KH, C], f32r)
            for k in range(KH):
                nc.sync.dma_start(out=w2s[:, k, :], in_=w2_r[:, k, :].bitcast(f32r))
            probs = const.tile([128, n_tiles], f32)
            nc.sync.dma_start(out=probs, in_=prob_r)
            eps = const.tile([128, 1], f32)
            nc.vector.memset(eps, LN_EPS)
            ident = const.tile([128, 128], f32)
            make_identity(nc, ident)
            if use_b1:
                b1s = const.tile([128, H], f32)
                nc.sync.dma_start(out=b1s, in_=b1.to_broadcast([128, H]))
            if use_ln_affine:
                gs = const.tile([128, H], f32)
                bs = const.tile([128, H], f32)
                nc.sync.dma_start(out=gs, in_=ln_g.to_broadcast([128, H]))
                nc.sync.dma_start(out=bs, in_=ln_b.to_broadcast([128, H]))

            n_groups = (n_tiles + G - 1) // G
            for g in range(n_groups):
                tiles = [t for t in range(g * G, min((g + 1) * G, n_tiles))]
                xts = {}
                for t in tiles:
                    xts[t] = xp.tile([128, KD, 128], f32r, tag="xts", name="xts")
                    nc.gpsimd.dma_start(
                        out=xts[t], in_=xT_r[:, :, t * 128:(t + 1) * 128].bitcast(f32r)
                    )
                h_sb = {t: hp.tile([128, H], f32, tag="h", name="h") for t in tiles}
                stats = {t: stp.tile([128, HJ, 6], f32, tag="st", name="st") for t in tiles}
                # ---- matmul1, streamed over H chunks of 512 ----
                for j in range(HJ):
                    w1c = w1p.tile([128, KD, 512], f32r, tag="w1c")
                    for k in range(KD):
                        nc.gpsimd.dma_start(
                            out=w1c[:, k, :],
                            in_=w1_r[:, k, j * 512:(j + 1) * 512].bitcast(f32r),
                        )
                    for t in tiles:
                        ph = hps.tile([128, 512], f32, tag="hps")
                        for k in range(KD):
                            nc.tensor.matmul(
                                ph, xts[t][:, k, :], w1c[:, k, :],
                                start=(k == 0), stop=(k == KD - 1),
                            )
                        js = slice(j * 512, (j + 1) * 512)
                        if use_b1:
                            # h = ph + b1 (DVE), stats read the biased h
                            nc.vector.tensor_add(h_sb[t][:, js], ph, b1s[:, js])
                            nc.vector.bn_stats(stats[t][:, j, :], h_sb[t][:, js])
                        else:
                            nc.vector.bn_stats(stats[t][:, j, :], ph)
                            nc.scalar.copy(h_sb[t][:, js], ph)
                # ---- LayerNorm + GELU + transpose + matmul2, per tile ----
                for t in tiles:
                    mv = stp.tile([128, 2], f32, tag="mv")
                    nc.vector.bn_aggr(mv, stats[t])
                    std = stp.tile([128, 1], f32, tag="std")
                    nc.scalar.activation(std, mv[:, 1:2], AF.Sqrt, bias=eps)
                    rstd = stp.tile([128, 1], f32, tag="rstd")
                    nc.vector.reciprocal(rstd, std)
                    nc.vector.tensor_scalar(
                        h_sb[t], h_sb[t], mv[:, 0:1], None,
                        mybir.AluOpType.subtract,
                    )
                    if use_ln_affine:
                        # v = v*rstd (per-token), then v = v*g + b (per-H)
                        nc.scalar.activation(h_sb[t], h_sb[t], AF.Copy, scale=rstd)
                        nc.vector.tensor_mul(h_sb[t], h_sb[t], gs)
                        nc.vector.tensor_add(h_sb[t], h_sb[t], bs)
                        nc.scalar.activation(h_sb[t], h_sb[t], AF.Gelu)
                    else:
                        nc.scalar.activation(
                            h_sb[t], h_sb[t], AF.Gelu, scale=rstd
                        )
                    # transpose u -> uT (PE transpose, 4 cols per PSUM bank)
                    uT = utp.tile([128, KH, 128], f32r, tag="uT")
                    for cb in range(KH // 4):
                        tp = tps.tile([128, 512], f32, tag="tps")
                        for q in range(4):
                            c = cb * 4 + q
                            nc.tensor.transpose(
                                tp[:, q * 128:(q + 1) * 128],
                                h_sb[t][:, c * 128:(c + 1) * 128],
                                ident,
                            )
                        nc.vector.tensor_copy(uT[:, cb * 4:(cb + 1) * 4, :], tp)
                    # matmul2: y[t] = u @ w2  (contract over H)
                    py = yps.tile([128, C], f32, tag="yps")
                    for k in range(KH):
                        nc.tensor.matmul(
                            py, uT[:, k, :], w2s[:, k, :],
                            start=(k == 0), stop=(k == KH - 1),
                        )
                    ysb = ypool.tile([128, C], f32, tag="ysb")
                    nc.scalar.activation(
                        ysb, py, AF.Copy, scale=probs[:, t:t + 1]
                    )
                    nc.sync.dma_start(out=y_r[t], in_=ysb)

    nc.compile()
    return nc


def _route_host(x, gate_w, gate_b):
    """Router via the same jax ops as the reference, on the process-default
    backend, so logits/top-k bit-match the reference run in this process."""
    import jax
    import jax.numpy as jnp

    xl = jnp.asarray(x)
    logits = xl @ jnp.asarray(gate_w) + jnp.asarray(gate_b)
    top_vals, top_idx = jax.lax.top_k(logits, K)
    probs = jax.nn.softmax(top_vals, axis=1)
    return (
        np.asarray(logits),
        np.asarray(top_idx),
        np.asarray(probs).astype(np.float32),
    )


def kernel(x, gate_w, gate_b, w1, b1, ln_g, ln_b, w2, b2):
    from concourse.bass_utils import run_bass_kernel_spmd

    x = np.ascontiguousarray(np.asarray(x, dtype=np.float32))
    logits, top_idx, probs = _route_host(x, np.asarray(gate_w), np.asarray(gate_b))

    use_b1 = bool(np.any(np.asarray(b1)))
    use_ln_affine = bool(
        np.any(np.asarray(ln_g) != 1.0) or np.any(np.asarray(ln_b))
    )

    # dispatch: token indices + gate probs per expert
    idx_e, prob_e = [], []
    e0, e1 = top_idx[:, 0], top_idx[:, 1]
    p0, p1 = probs[:, 0], probs[:, 1]
    for e in range(E):
        i0 = np.nonzero(e0 == e)[0]
        i1 = np.nonzero(e1 == e)[0]
        idx_e.append(np.concatenate([i0, i1]))
        prob_e.append(np.concatenate([p0[i0], p1[i1]]).astype(np.float32))
    max_n = max(len(i) for i in idx_e)
    n_tiles = -(-max_n // 256) * 2  # multiple of 2 tiles (G=2)
    N = n_tiles * 128

    key = (n_tiles, use_b1, use_ln_affine)
    if key not in _cache:
        _cache[key] = _build(*key)
    nc = _cache[key]

    xT_full = np.ascontiguousarray(x.T)  # [D, B]
    in_maps = []
    w1_np = np.ascontiguousarray(np.asarray(w1, dtype=np.float32))
    w2_np = np.ascontiguousarray(np.asarray(w2, dtype=np.float32))
    b1_np = np.asarray(b1, dtype=np.float32)
    for e in range(E):
        n_e = len(idx_e[e])
        xTe = np.zeros((D, N), dtype=np.float32)
        xTe[:, :n_e] = xT_full[:, idx_e[e]]
        pe = np.zeros((N,), dtype=np.float32)
        pe[:n_e] = prob_e[e]
        m = {"xT": xTe, "w1": w1_np[e], "w2": w2_np[e], "prob": pe}
        if use_b1:
            m["b1"] = np.ascontiguousarray(b1_np[e])
        if use_ln_affine:
            m["ln_g"] = np.ascontiguousarray(np.asarray(ln_g, np.float32)[e])
            m["ln_b"] = np.ascontiguousarray(np.asarray(ln_b, np.float32)[e])
        in_maps.append(m)

    res = run_bass_kernel_spmd(nc, in_maps, core_ids=list(range(E)))

    out = np.zeros((B, C), dtype=np.float32)
    for e in range(E):
        n_e = len(idx_e[e])
        out[idx_e[e]] += res.results[e]["y"][:n_e]
    b2_np = np.asarray(b2, dtype=np.float32)
    if np.any(b2_np):
        out += p0[:, None] * b2_np[e0] + p1[:, None] * b2_np[e1]
    return out, logits.astype(np.float32)
